# revision 1
# baseline (speedup 1.0000x reference)
"""Single-token GQA decode attention (32 q heads / 8 kv heads, 8192-pos KV
cache, dim 4096) tensor-parallel over 8 NeuronCores.

Sharding (per core c): q heads [4c, 4c+4), kv head c; x replicated; each core
emits a full-width [1, 4096] output partial, summed host-side.

Schedule: the three DMA queues (SP HWDGE, ACT HWDGE, Pool SWDGE) stream
disjoint slices of the weights/KV concurrently, balanced to finish the
attention inputs (wqkv, K, V) simultaneously; the wo stream follows, cycled
across queues j-major so the output-projection matmuls are never starved.
All pre-output compute is transposed on the PE so it is nearly free:

  - q/k/v projection: per (chunk, head) matmul with out [128, 1] psum col,
    contracting x chunks over partitions (192 matmuls, ~0ns each).
  - RoPE: one [128, 5] matmul against a host-built 128x128 block-diagonal
    rotation matrix (exact same math as the reference's complex rotation).
  - scores/AV: [128, 4]-out matmuls per 128-position chunk, exp on ACT with
    the 1/sqrt(d) scale folded in (groups aligned to K-piece arrival),
    normalization via ones-matmul column sums + reciprocal broadcast,
    fully computed before the last V bytes land.
  - output projection: [1, N]-out matmuls (4 head passes per col block)
    chased by the wo stream; PSUM drained to SBUF per block (DVE mostly,
    Pool/ACT for the last blocks so nothing queues behind the DVE chain)
    and shipped to DRAM in a few range DMAs. The final block is small and
    drains through the otherwise-idle ACT so the stop->drain->out chain on
    the critical tail is minimal; the two ending chains (SP bulk piece,
    ACT final piece) end at exactly the same tick.

Weights/KV move as fp16 (error ~1e-3 vs fp32 reference); matmul accumulation
is fp32 in PSUM; softmax statistics fp32.
"""

import numpy as np

import concourse.tile as tile
from concourse import bacc, mybir
from concourse.bass_utils import run_bass_kernel_spmd
from concourse.tile import add_dep_helper

N_CORES = 8
DIM = 4096
HEAD_DIM = 128
N_HEADS = 32
N_KV_HEADS = 8
REPEATS = N_HEADS // N_KV_HEADS  # 4 q heads per core
KV_LEN = 8192                    # start_pos + 1
KCH = DIM // 128                 # 32 contraction chunks
TCH = KV_LEN // 128              # 64 kv-position chunks
QCOLS = REPEATS * 128            # 512
WCOLS = QCOLS + 2 * 128          # 768 merged qkv cols per chunk
XTRA = KCH + 256                 # x (32) + rot/id (256) cols on queue 0
SCALE = 1.0 / np.sqrt(np.float32(HEAD_DIM))

F32 = mybir.dt.float32
F16 = mybir.dt.float16

# ---- stream split (tunable) -------------------------------------------------
# wqkv chunk ranges per queue (SP also carries x + rot/id appended)
W_SPLIT = [(0, 12), (12, 21), (21, 32)]
# kt column pieces (queue, lo, hi); SP's piece contains the new-k slot 8064
KT_PIECES = [(0, 6144, 8192), (1, 0, 3328), (2, 3328, 6144)]
# v chunk pieces (queue, lo, hi); chunk 63 first on its queue (new-v WAW)
V_PIECES = [(0, 48, 64), (0, 0, 5), (1, 5, 23), (2, 23, 48)]
# exp/score groups aligned to kt pieces, ordered by expected arrival
SCORE_GROUPS = [(48, 64), (0, 26), (26, 48)]
# output-projection blocks in CONSUMPTION order (psum col lo, hi).
# banks 6 and 7 are split, with their first halves consumed FIRST (their
# drains are long done before the later blocks reuse those banks) and a
# tiny 64-col block LAST so the drain on the critical tail is minimal.
O_BLOCKS = ([(3584, 3840)]
            + [(j * 512, (j + 1) * 512) for j in range(6)]
            + [(3840, 3904), (3072, 3584), (3904, 4096)])
# wo stream piece widths (flat consumption-order cols): small early pieces
# for low fill latency, 1024-wide later for DMA efficiency
WO_PIECE_COLS = [512] * 10 + [1024] * 11

_CACHED = {}


def _build():
    nc = bacc.Bacc(None, target_bir_lowering=False)

    # per-queue wqkv streams, host-flattened to [128, cols]
    wq_d = [
        nc.dram_tensor(f"wq{q}", [128, (c1 - c0) * WCOLS + (XTRA if q == 0 else 0)],
                       F16, kind="ExternalInput")
        for q, (c0, c1) in enumerate(W_SPLIT)
    ]
    k_t = nc.dram_tensor("k_t", [128, KV_LEN], F16, kind="ExternalInput")
    v_s = nc.dram_tensor("v_s", [128, TCH, 128], F16, kind="ExternalInput")
    wo_t = nc.dram_tensor("wo_t", [128, REPEATS * DIM], F16, kind="ExternalInput")
    out_p = nc.dram_tensor("out_p", [1, DIM], F32, kind="ExternalOutput")

    tails = [None, None, None, None, None]

    def chain(q, inst):
        if tails[q] is not None:
            add_dep_helper(inst.ins, tails[q].ins, sync=False,
                           reason="stream order")
        tails[q] = inst

    with tile.TileContext(nc) as tc:
        with (
            tc.tile_pool(name="big", bufs=1) as big,
            tc.tile_pool(name="small", bufs=1) as small,
        ):
            engs = [nc.sync, nc.scalar, nc.gpsimd]

            # w_sb column layout: queue-1 chunks, queue-2 chunks, queue-0
            # chunks, then x + rot/id -- so every queue's slice (incl. the
            # extras riding on queue 0) is one contiguous DMA dest
            w_sb = big.tile([128, KCH * WCOLS + XTRA], F16)
            wbase = {}
            off = 0
            for q in (1, 2, 0):
                c0, c1 = W_SPLIT[q]
                for c in range(c0, c1):
                    wbase[c] = off
                    off += WCOLS
            w_off = {q: wbase[W_SPLIT[q][0]] for q in range(3)}
            kt_sb = big.tile([128, KV_LEN], F16)
            v_sb = big.tile([128, TCH, 128], F16)
            wo_sb = big.tile([128, REPEATS * DIM], F16)

            x_sb = w_sb[:, KCH * WCOLS : KCH * WCOLS + KCH]
            rot_sb = w_sb[:, KCH * WCOLS + KCH : KCH * WCOLS + KCH + 128]
            id_sb = w_sb[:, KCH * WCOLS + KCH + 128 : KCH * WCOLS + XTRA]

            qk_sb = small.tile([128, 6], F16)
            qT = small.tile([128, REPEATS], F16)
            attn = small.tile([128, REPEATS], F16)
            e_sb = small.tile([128, TCH * REPEATS], F16)
            zps = [small.tile([128, REPEATS], F32, name=f"zp{g}")
                   for g in range(len(SCORE_GROUPS))]
            rz_sb = small.tile([1, REPEATS], F32)
            rzb_sb = small.tile([128, REPEATS], F32)
            ones_sb = small.tile([128, 1], F32)
            ones_row = small.tile([1, 128], F32)

            nc.vector.memset(ones_sb[:], 1.0)
            nc.vector.memset(ones_row[:], 1.0)

            # --- wqkv streams (x + rot/id ride along on queue 0) ---
            for q, (c0, c1) in enumerate(W_SPLIT):
                cols = (c1 - c0) * WCOLS + (XTRA if q == 0 else 0)
                lo = w_off[q]
                chain(q, engs[q].dma_start(
                    out=w_sb[:, lo : lo + cols], in_=wq_d[q][:]))
            # --- K / V cache streams ---
            for q, lo, hi in KT_PIECES:
                chain(q, engs[q].dma_start(
                    out=kt_sb[:, lo:hi], in_=k_t[:, lo:hi]))
            v_dmas = {}
            for q, lo, hi in V_PIECES:
                v_dmas[(lo, hi)] = engs[q].dma_start(
                    out=v_sb[:, lo:hi, :], in_=v_s[:, lo:hi, :])
                chain(q, v_dmas[(lo, hi)])

            with tc.tile_pool(name="ps_ab", bufs=1, space="PSUM") as ps_ab:
                pqkv = ps_ab.tile([128, 6], F32)
                prot = ps_ab.tile([128, 5], F32)
                pvrow = ps_ab.tile([1, 128], F32)

                # transposed qkv projection: out cols [q0 q1 q2 q3 k v];
                # col groups sequential (one psum accum group per bank)
                for col in range(6):
                    for c in range(KCH):
                        base = wbase[c]
                        nc.tensor.matmul(
                            pqkv[:, col : col + 1],
                            w_sb[:, base + col * 128 : base + (col + 1) * 128],
                            x_sb[:, c : c + 1],
                            start=(c == 0), stop=(c == KCH - 1),
                        )
                nc.vector.tensor_copy(qk_sb[:], pqkv[:])
                # RoPE on q cols + k col in one matmul; v passes through
                nc.tensor.matmul(prot[:], rot_sb, qk_sb[:, 0:5],
                                 start=True, stop=True)
                nc.vector.tensor_copy(qT[:], prot[:, 0:REPEATS])
                # chunk 63's position slots are rotated host-side so the new
                # position (8191) sits at slot 0 -> partition-0 writes
                kb = (TCH - 1) * 128
                nc.vector.tensor_copy(
                    kt_sb[:, kb : kb + 1], prot[:, REPEATS : REPEATS + 1])
                # new-v row via identity matmul ([128,1] col -> [1,128] row)
                nc.tensor.matmul(pvrow[:], qk_sb[:, 5:6], id_sb,
                                 start=True, stop=True)

                pscore = ps_ab.tile([128, TCH * REPEATS], F32)
                pav = ps_ab.tile([128, REPEATS], F32)
                pz = ps_ab.tile([1, REPEATS], F32)
                przb = ps_ab.tile([128, REPEATS], F32)

                # scores_T [128 t, 4 h] per chunk, in kt arrival order
                for jlo, jhi in SCORE_GROUPS:
                    for j in range(jlo, jhi):
                        chain(4, nc.tensor.matmul(
                            pscore[:, j * REPEATS : (j + 1) * REPEATS],
                            kt_sb[:, j * 128 : (j + 1) * 128],
                            qT[:],
                            start=True, stop=True,
                        ))
                # one exp over all scores on ACT (chained after ACT's v
                # pieces; every kt piece lands by ~9.4us so all scores are
                # ready when ACT frees up), then z -> 1/z -> broadcast
                ev = e_sb[:].rearrange("p (j h) -> p h j", h=REPEATS)
                chain(1, nc.scalar.activation(
                    e_sb[:], pscore[:],
                    mybir.ActivationFunctionType.Exp,
                    scale=float(SCALE),
                ))
                chain(3, nc.vector.reduce_sum(
                    zps[0][:], ev[:], axis=mybir.AxisListType.X))
                chain(4, nc.tensor.matmul(pz[:], ones_sb[:], zps[0][:],
                                           start=True, stop=True))
                # v-row scatter after the zp reduces (which are ready
                # earlier) but before the reciprocal (whose pz input is
                # still in flight on the PE at that point)
                chain(3, nc.vector.tensor_copy(v_sb[0:1, TCH - 1, :], pvrow[:]))
                chain(3, nc.vector.reciprocal(rz_sb[:], pz[:]))
                chain(4, nc.tensor.matmul(przb[:], ones_row[:], rz_sb[:],
                                           start=True, stop=True))
                # engines can read only ONE psum operand per op: stage the
                # broadcast reciprocal in SBUF (DVE, slotted between the
                # reciprocal and the attn multiply)
                chain(3, nc.vector.tensor_copy(rzb_sb[:], przb[:]))

                # AV in v arrival order; chunk 63 last (new-v scatter WAW)
                av_order = []
                for _q, lo, hi in V_PIECES:
                    av_order += list(range(lo, hi))
                av_order.remove(TCH - 1)
                av_order.append(TCH - 1)
                for idx, j in enumerate(av_order):
                    chain(4, nc.tensor.matmul(
                        pav[:], v_sb[:, j, :],
                        e_sb[:, j * REPEATS : (j + 1) * REPEATS],
                        start=(idx == 0), stop=(idx == TCH - 1),
                    ))
                chain(3, nc.vector.tensor_mul(attn[:], pav[:], rzb_sb[:]))

            # --- wo stream: consumption-order flat layout, pieces cycled
            # across the three queues ---
            off = 0
            for i, w in enumerate(WO_PIECE_COLS):
                q = i % 3
                chain(q, engs[q].dma_start(
                    out=wo_sb[:, off : off + w], in_=wo_t[:, off : off + w]))
                off += w
            assert off == REPEATS * DIM

            with tc.tile_pool(name="ps_o", bufs=1, space="PSUM") as ps_o:
                pout = ps_o.tile([1, DIM], F32)
                o_e = small.tile([1, DIM], F32)    # DVE drain rows
                o_t = small.tile([1, 192], F32)    # final block (ACT drain)
                last = len(O_BLOCKS) - 1
                flat = 0
                for bi, (lo, hi) in enumerate(O_BLOCKS):
                    width = hi - lo
                    for h in range(REPEATS):
                        nc.tensor.matmul(
                            pout[:, lo:hi],
                            attn[:, h : h + 1],
                            wo_sb[:, flat + h * width : flat + (h + 1) * width],
                            start=(h == 0), stop=(h == REPEATS - 1),
                        )
                    flat += REPEATS * width
                    # drain each block right after its stop matmul; DVE
                    # takes all but the tiny final block, which goes to ACT
                    # (idle by then) into its own tile so it never queues
                    # behind the DVE drain chain
                    if bi == last:
                        nc.scalar.copy(o_t[:], pout[:, lo:hi])
                    else:
                        nc.vector.tensor_copy(o_e[:, lo:hi], pout[:, lo:hi])
                    if (lo, hi) == (1536, 2048):
                        chain(0, nc.sync.dma_start(
                            out=out_p[:, 0:2048], in_=o_e[:, 0:2048]))
                    elif (lo, hi) == (3584, 4032):
                        pass
                # remaining pieces in drain-completion order on SP; the final
                # 64-col piece rides ACT right behind its drain
                chain(0, nc.sync.dma_start(
                    out=out_p[:, 2048:3904], in_=o_e[:, 2048:3904]))
                chain(1, nc.scalar.dma_start(
                    out=out_p[:, 3904:DIM], in_=o_t[:]))

    nc.compile()
    return nc


def _shard_inputs(x, wq, wk, wv, wo, cache_k, cache_v, cos, sin):
    """Build the 8 per-core input maps (fp16 weights/KV, C-contiguous)."""
    x_flat = np.asarray(x, dtype=np.float32).reshape(DIM)
    x_col = x_flat.reshape(KCH, 128).T.astype(np.float16)  # [128, 32]

    cos = np.asarray(cos, np.float32).reshape(-1)  # [64]
    sin = np.asarray(sin, np.float32).reshape(-1)
    # rot = R.T (matmul lhsT layout) for the block-diag 2x2 rotation R;
    # id = 128x128 identity for the column->row transpose matmul
    rot = np.zeros((128, 128), np.float32)
    i = np.arange(64)
    rot[2 * i, 2 * i] = cos
    rot[2 * i + 1, 2 * i + 1] = cos
    rot[2 * i + 1, 2 * i] = -sin
    rot[2 * i, 2 * i + 1] = sin
    xtra = np.concatenate(
        [x_col, rot.astype(np.float16), np.eye(128, dtype=np.float16)], axis=1)

    wq = np.asarray(wq, np.float32)
    wk = np.asarray(wk, np.float32)
    wv = np.asarray(wv, np.float32)
    wo = np.asarray(wo, np.float32)
    cache_k = np.asarray(cache_k, np.float32)
    cache_v = np.asarray(cache_v, np.float32)

    in_maps = []
    for c in range(N_CORES):
        wq_c = wq[c * QCOLS : (c + 1) * QCOLS]             # [512, 4096]
        wk_c = wk[c * HEAD_DIM : (c + 1) * HEAD_DIM]       # [128, 4096]
        wv_c = wv[c * HEAD_DIM : (c + 1) * HEAD_DIM]
        q_blk = (wq_c.reshape(REPEATS, 128, KCH, 128)
                 .transpose(2, 3, 0, 1).reshape(KCH, 128, QCOLS))
        k_blk = wk_c.reshape(128, KCH, 128).transpose(1, 2, 0)
        v_blk = wv_c.reshape(128, KCH, 128).transpose(1, 2, 0)
        wqkv_c = np.concatenate([q_blk, k_blk, v_blk], axis=2)  # [32,128,768]
        wqkv_c = wqkv_c.transpose(1, 0, 2).reshape(128, KCH * WCOLS)
        m = {}
        for q, (c0, c1) in enumerate(W_SPLIT):
            piece = wqkv_c[:, c0 * WCOLS : c1 * WCOLS]
            if q == 0:
                piece = np.concatenate([piece, xtra], axis=1)
            m[f"wq{q}"] = np.ascontiguousarray(piece.astype(np.float16))
        # chunk 63 slot rotation: slot 0 <- new position (device-written),
        # slots 1..127 <- cache positions 8064..8190
        kraw = cache_k[0, :KV_LEN, c, :].T  # [128, 8192]
        k_c = np.empty((128, KV_LEN), np.float16)
        k_c[:, : KV_LEN - 128] = kraw[:, : KV_LEN - 128]
        k_c[:, KV_LEN - 128] = 0
        k_c[:, KV_LEN - 127 :] = kraw[:, KV_LEN - 128 : KV_LEN - 1]
        m["k_t"] = np.ascontiguousarray(k_c)
        vraw = cache_v[0, :KV_LEN, c, :]  # [8192, 128]
        v_c = np.empty((TCH, 128, HEAD_DIM), np.float16)
        v_c[: TCH - 1] = vraw[: KV_LEN - 128].reshape(TCH - 1, 128, HEAD_DIM)
        v_c[TCH - 1, 0] = 0
        v_c[TCH - 1, 1:] = vraw[KV_LEN - 128 : KV_LEN - 1]
        m["v_s"] = np.ascontiguousarray(v_c.transpose(1, 0, 2))
        wo_hm = (wo[:, c * QCOLS : (c + 1) * QCOLS].T
                 .reshape(REPEATS, 128, DIM).transpose(1, 0, 2))  # [128,h,col]
        m["wo_t"] = np.ascontiguousarray(np.concatenate(
            [wo_hm[:, h, lo:hi] for (lo, hi) in O_BLOCKS
             for h in range(REPEATS)], axis=1).astype(np.float16))
        in_maps.append(m)
    return in_maps


def get_program(reps=1):
    if "nc" not in _CACHED:
        _CACHED["nc"] = _build()
    return _CACHED["nc"]


def kernel(x, wq, wk, wv, wo, cache_k, cache_v, cos, sin, start_pos):
    nc = get_program()
    in_maps = _shard_inputs(x, wq, wk, wv, wo, cache_k, cache_v, cos, sin)
    res = run_bass_kernel_spmd(nc, in_maps, list(range(N_CORES)))
    out = np.zeros((1, DIM), np.float32)
    for c in range(N_CORES):
        out += res.results[c]["out_p"]
    return out.reshape(1, 1, DIM)



# revision 2
# speedup vs baseline: 1.1273x; 1.1273x over previous
"""Single-token GQA decode attention (32 q heads / 8 kv heads, 8192-pos KV
cache, dim 4096) tensor-parallel over 8 NeuronCores.

Sharding (per core c): q heads [4c, 4c+4), kv head c; x replicated; each core
emits a [128, 32] column-chunked partial of its full-width [1, 4096] output
projection, summed + transposed host-side.

Schedule: three DMA queues (SP/ACT HWDGE, Pool SWDGE) each stream ONE mega
piece holding their share of {x/rot/id extras, wqkv, K^T, V} followed by their
share of the wo stream, balanced so all queues end together.  All attention
compute (q/k/v proj on the PE with [128,1] psum cols, RoPE via a host-built
block-diagonal rotation matmul, scores/exp/softmax-z, AV) runs while the wo
stream is still in flight, so the only exposed tail is: last wo block ->
4 matmuls -> psum drain -> one [128,32] f32 output DMA.

Output projection is TRANSPOSED on the PE: out^T[128 outs, 32 chunks] with
lhsT = wo block [128 contract-dim, 128 outs] and rhs = attn column [128, 1]
(free-dim-1 matmuls are ~free), accumulating 4 head blocks per out chunk.
The host undoes the [128, 32] chunk-major layout when summing partials.

Weights/KV move as fp16 (error ~1e-3 vs fp32 reference); matmul accumulation
is fp32 in PSUM; softmax statistics fp32.
"""

import numpy as np

import concourse.tile as tile
from concourse import bacc, mybir
from concourse.bass_utils import run_bass_kernel_spmd
from concourse.tile import add_dep_helper

N_CORES = 8
DIM = 4096
HEAD_DIM = 128
N_HEADS = 32
N_KV_HEADS = 8
REPEATS = N_HEADS // N_KV_HEADS  # 4 q heads per core
KV_LEN = 8192                    # start_pos + 1
KCH = DIM // 128                 # 32 contraction chunks
TCH = KV_LEN // 128              # 64 kv-position chunks
QCOLS = REPEATS * 128            # 512
WCOLS = QCOLS + 2 * 128          # 768 merged qkv cols per chunk
XTRA = KCH + 256                 # x (32) + rot (128) + id (128) cols
OCH = DIM // 128                 # 32 output col chunks
SCALE = 1.0 / np.sqrt(np.float32(HEAD_DIM))

F32 = mybir.dt.float32
F16 = mybir.dt.float16

# ---- stream split (tunable) -------------------------------------------------
# per queue: wqkv chunk range, kt chunk range, v chunk range, wo oc range.
# Queue 0 = SP (also carries extras + the tiny final wo block + out DMA),
# queue 1 = ACT (late start: act-table load), queue 2 = Pool (SWDGE).
W_SPLIT = [(0, 10), (10, 21), (21, 32)]
KT_SPLIT = [(39, 64), (0, 17), (17, 39)]   # q0 range contains chunk 63 (new k)
V_SPLIT = [(42, 64), (0, 16), (16, 42)]    # q0 range contains chunk 63 (new v)
WO_SPLIT = [(21, 32), (10, 21), (0, 10)]   # oc ranges; q0 last, ends w/ 1 blk

_CACHED = {}


def _mega_layout():
    """Per-queue column layout of the mega tile: maps for wqkv chunk, kt
    chunk, v chunk -> (queue, col offset).  Extras live at the start of q0."""
    wq_off, kt_off, v_off, mega_cols = {}, {}, {}, []
    for q in range(3):
        off = XTRA if q == 0 else 0
        for c in range(*W_SPLIT[q]):
            wq_off[c] = (q, off)
            off += WCOLS
        for j in range(*KT_SPLIT[q]):
            kt_off[j] = (q, off)
            off += 128
        for j in range(*V_SPLIT[q]):
            v_off[j] = (q, off)
            off += 128
        mega_cols.append(off)
    return wq_off, kt_off, v_off, mega_cols


def _build():
    nc = bacc.Bacc(None, target_bir_lowering=False)

    wq_off, kt_off, v_off, mega_cols = _mega_layout()
    wo_cols = [(hi - lo) * REPEATS * 128 for lo, hi in WO_SPLIT]

    s_d = [nc.dram_tensor(f"s{q}", [128, mega_cols[q]], F16, kind="ExternalInput")
           for q in range(3)]
    wo_d = [nc.dram_tensor(f"wo{q}", [128, wo_cols[q]], F16, kind="ExternalInput")
            for q in range(3)]
    out_p = nc.dram_tensor("out_p", [128, OCH], F32, kind="ExternalOutput")

    tails = [None, None, None]

    def chain(q, inst):
        if tails[q] is not None:
            add_dep_helper(inst.ins, tails[q].ins, sync=False,
                           reason="stream order")
        tails[q] = inst

    with tile.TileContext(nc) as tc:
        with (
            tc.tile_pool(name="big", bufs=1) as big,
            tc.tile_pool(name="small", bufs=1) as small,
        ):
            engs = [nc.sync, nc.scalar, nc.gpsimd]

            sb = [big.tile([128, mega_cols[q]], F16, name=f"sb{q}")
                  for q in range(3)]
            wo_sb = [big.tile([128, wo_cols[q]], F16, name=f"wosb{q}")
                     for q in range(3)]

            x_sb = sb[0][:, 0:KCH]
            rot_sb = sb[0][:, KCH:KCH + 128]
            id_sb = sb[0][:, KCH + 128:XTRA]

            def wblk(c, col):     # wqkv chunk c, inner col block [128]
                q, off = wq_off[c]
                return sb[q][:, off + col * 128: off + (col + 1) * 128]

            def ktblk(j):
                q, off = kt_off[j]
                return sb[q][:, off:off + 128]

            def vblk(j):
                q, off = v_off[j]
                return sb[q][:, off:off + 128]

            def woblk(oc, h):
                for q, (lo, hi) in enumerate(WO_SPLIT):
                    if lo <= oc < hi:
                        off = ((oc - lo) * REPEATS + h) * 128
                        return wo_sb[q][:, off:off + 128]
                raise AssertionError

            qk_sb = small.tile([128, 6], F16)
            qT = small.tile([128, REPEATS], F16)
            attn = small.tile([128, REPEATS], F16)
            e_sb = small.tile([128, TCH * REPEATS], F16)
            zp_sb = small.tile([128, REPEATS], F32)
            rz_sb = small.tile([1, REPEATS], F32)
            rzb_sb = small.tile([128, REPEATS], F32)
            ones_sb = small.tile([128, 1], F32)
            ones_row = small.tile([1, 128], F32)
            o_sb = small.tile([128, OCH], F32)

            nc.vector.memset(ones_sb[:], 1.0)
            nc.vector.memset(ones_row[:], 1.0)

            # --- streams: one mega DMA per queue, then the wo stream ---
            for q in range(3):
                chain(q, engs[q].dma_start(out=sb[q][:], in_=s_d[q][:]))
            for q in (2, 1):
                chain(q, engs[q].dma_start(out=wo_sb[q][:], in_=wo_d[q][:]))
            # q0 wo: main piece + tiny final block so the exposed tail after
            # the very last input byte is minimal
            w0 = wo_cols[0]
            chain(0, engs[0].dma_start(
                out=wo_sb[0][:, 0:w0 - 128], in_=wo_d[0][:, 0:w0 - 128]))
            chain(0, engs[0].dma_start(
                out=wo_sb[0][:, w0 - 128:w0], in_=wo_d[0][:, w0 - 128:w0]))

            with tc.tile_pool(name="ps", bufs=1, space="PSUM") as ps:
                pqkv = ps.tile([128, 6], F32)
                prot = ps.tile([128, 5], F32)
                pvrow = ps.tile([1, 128], F32)
                pscore = ps.tile([128, TCH * REPEATS], F32)
                pav = ps.tile([128, REPEATS], F32)
                pz = ps.tile([1, REPEATS], F32)
                przb = ps.tile([128, REPEATS], F32)
                pout = ps.tile([128, OCH], F32)

                # qkv projection, transposed: psum cols [q0 q1 q2 q3 k v]
                for col in range(6):
                    for c in range(KCH):
                        nc.tensor.matmul(
                            pqkv[:, col:col + 1],
                            wblk(c, col),
                            x_sb[:, c:c + 1],
                            start=(c == 0), stop=(c == KCH - 1),
                        )
                nc.vector.tensor_copy(qk_sb[:], pqkv[:])
                # RoPE on q cols + k col in one matmul; v passes through
                nc.tensor.matmul(prot[:], rot_sb, qk_sb[:, 0:5],
                                 start=True, stop=True)
                nc.vector.tensor_copy(qT[:], prot[:, 0:REPEATS])
                # chunk 63's position slots are rotated host-side so the new
                # position (8191) sits at slot 0 -> col 0 of kt chunk 63
                nc.vector.tensor_copy(
                    ktblk(TCH - 1)[:, 0:1], prot[:, REPEATS:REPEATS + 1])
                # new-v row via identity matmul ([128,1] col -> [1,128] row)
                nc.tensor.matmul(pvrow[:], qk_sb[:, 5:6], id_sb,
                                 start=True, stop=True)
                # scatter new v into partition 0 of v chunk 63
                vt = vblk(TCH - 1)
                nc.vector.tensor_copy(vt[0:1, :], pvrow[:])

                # scores_T [128 t, 4 h] per chunk
                for j in range(TCH):
                    nc.tensor.matmul(
                        pscore[:, j * REPEATS:(j + 1) * REPEATS],
                        ktblk(j), qT[:], start=True, stop=True)
                # exp on ACT (emitted after ACT's DMAs so its sem wait never
                # delays the wo stream), then z -> 1/z -> broadcast
                ev = e_sb[:].rearrange("p (j h) -> p h j", h=REPEATS)
                nc.scalar.activation(
                    e_sb[:], pscore[:],
                    mybir.ActivationFunctionType.Exp, scale=float(SCALE))
                nc.vector.reduce_sum(zp_sb[:], ev[:], axis=mybir.AxisListType.X)
                nc.tensor.matmul(pz[:], ones_sb[:], zp_sb[:],
                                 start=True, stop=True)
                nc.vector.reciprocal(rz_sb[:], pz[:])
                nc.tensor.matmul(przb[:], ones_row[:], rz_sb[:],
                                 start=True, stop=True)
                nc.vector.tensor_copy(rzb_sb[:], przb[:])

                # AV; chunk 63 last (new-v row WAW)
                av_order = [j for j in range(TCH - 1)] + [TCH - 1]
                for idx, j in enumerate(av_order):
                    nc.tensor.matmul(
                        pav[:], vblk(j),
                        e_sb[:, j * REPEATS:(j + 1) * REPEATS],
                        start=(idx == 0), stop=(idx == TCH - 1),
                    )
                nc.vector.tensor_mul(attn[:], pav[:], rzb_sb[:])

                # transposed output projection: out^T[:, oc] accumulates 4
                # head blocks; free-dim-1 matmuls are ~free on the PE
                for oc in range(OCH):
                    for h in range(REPEATS):
                        nc.tensor.matmul(
                            pout[:, oc:oc + 1],
                            woblk(oc, h),
                            attn[:, h:h + 1],
                            start=(h == 0), stop=(h == REPEATS - 1),
                        )
                nc.vector.tensor_copy(o_sb[:], pout[:])
                chain(0, nc.sync.dma_start(out=out_p[:], in_=o_sb[:]))

    nc.compile()
    return nc


def _shard_inputs(x, wq, wk, wv, wo, cache_k, cache_v, cos, sin):
    """Build the 8 per-core input maps (fp16 weights/KV, C-contiguous)."""
    wq_off, kt_off, v_off, mega_cols = _mega_layout()

    x_flat = np.asarray(x, dtype=np.float32).reshape(DIM)
    x_col = x_flat.reshape(KCH, 128).T.astype(np.float16)  # [128, 32]

    cos = np.asarray(cos, np.float32).reshape(-1)  # [64]
    sin = np.asarray(sin, np.float32).reshape(-1)
    # rot = R.T (matmul lhsT layout) for the block-diag 2x2 rotation R
    rot = np.zeros((128, 128), np.float32)
    i = np.arange(64)
    rot[2 * i, 2 * i] = cos
    rot[2 * i + 1, 2 * i + 1] = cos
    rot[2 * i + 1, 2 * i] = -sin
    rot[2 * i, 2 * i + 1] = sin
    xtra = np.concatenate(
        [x_col, rot.astype(np.float16), np.eye(128, dtype=np.float16)], axis=1)

    wq = np.asarray(wq, np.float32)
    wk = np.asarray(wk, np.float32)
    wv = np.asarray(wv, np.float32)
    wo = np.asarray(wo, np.float32)
    cache_k = np.asarray(cache_k, np.float32)
    cache_v = np.asarray(cache_v, np.float32)

    in_maps = []
    for c in range(N_CORES):
        wq_c = wq[c * QCOLS:(c + 1) * QCOLS]              # [512, 4096]
        wk_c = wk[c * HEAD_DIM:(c + 1) * HEAD_DIM]        # [128, 4096]
        wv_c = wv[c * HEAD_DIM:(c + 1) * HEAD_DIM]
        q_blk = (wq_c.reshape(REPEATS, 128, KCH, 128)
                 .transpose(2, 3, 0, 1).reshape(KCH, 128, QCOLS))
        k_blk = wk_c.reshape(128, KCH, 128).transpose(1, 2, 0)
        v_blk = wv_c.reshape(128, KCH, 128).transpose(1, 2, 0)
        wqkv_c = np.concatenate([q_blk, k_blk, v_blk], axis=2)  # [32,128,768]
        wqkv_c = wqkv_c.astype(np.float16)
        # chunk 63 slot rotation: slot 0 <- new position (device-written),
        # slots 1..127 <- cache positions 8064..8190
        kraw = cache_k[0, :KV_LEN, c, :].T  # [128, 8192]
        k_c = np.empty((128, KV_LEN), np.float16)
        k_c[:, :KV_LEN - 128] = kraw[:, :KV_LEN - 128]
        k_c[:, KV_LEN - 128] = 0
        k_c[:, KV_LEN - 127:] = kraw[:, KV_LEN - 128:KV_LEN - 1]
        vraw = cache_v[0, :KV_LEN, c, :]  # [8192, 128]
        v_c = np.empty((TCH, 128, HEAD_DIM), np.float16)
        v_c[:TCH - 1] = vraw[:KV_LEN - 128].reshape(TCH - 1, 128, HEAD_DIM)
        v_c[TCH - 1, 0] = 0
        v_c[TCH - 1, 1:] = vraw[KV_LEN - 128:KV_LEN - 1]
        v_c = v_c.transpose(1, 0, 2)  # [128, 64, 128]

        m = {}
        for q in range(3):
            parts = []
            if q == 0:
                parts.append(xtra)
            for cc in range(*W_SPLIT[q]):
                parts.append(wqkv_c[cc])
            lo, hi = KT_SPLIT[q]
            parts.append(k_c[:, lo * 128:hi * 128])
            lo, hi = V_SPLIT[q]
            parts.append(v_c[:, lo:hi].reshape(128, (hi - lo) * 128))
            m[f"s{q}"] = np.ascontiguousarray(np.concatenate(parts, axis=1))
            assert m[f"s{q}"].shape[1] == mega_cols[q]
        wo_c = wo[:, c * QCOLS:(c + 1) * QCOLS].astype(np.float16)  # [4096,512]
        for q, (lo, hi) in enumerate(WO_SPLIT):
            blocks = [wo_c[oc * 128:(oc + 1) * 128, h * 128:(h + 1) * 128].T
                      for oc in range(lo, hi) for h in range(REPEATS)]
            m[f"wo{q}"] = np.ascontiguousarray(np.concatenate(blocks, axis=1))
        in_maps.append(m)
    return in_maps


def get_program(reps=1):
    if "nc" not in _CACHED:
        _CACHED["nc"] = _build()
    return _CACHED["nc"]


def kernel(x, wq, wk, wv, wo, cache_k, cache_v, cos, sin, start_pos):
    nc = get_program()
    in_maps = _shard_inputs(x, wq, wk, wv, wo, cache_k, cache_v, cos, sin)
    res = run_bass_kernel_spmd(nc, in_maps, list(range(N_CORES)))
    out = np.zeros(DIM, np.float32)
    for c in range(N_CORES):
        out += res.results[c]["out_p"].T.reshape(DIM)
    return out.reshape(1, 1, DIM)
